# revision 19
# baseline (speedup 1.0000x reference)
"""Trainium2 Bass kernel for nn_Attention (dense transformer attention over 32x32 fmap).

Math (per batch):
    qkv = w_qkv @ fmap_flat            # [1536, 1024] = [1536,512] @ [512,1024]
    q, k, v per head: [128, 1024] in (d, s) layout
    emb[s, d] = height[x] + width[y];  s = 32*x + y
    sim = (q^T (k + emb^T)) * scale    # scale folded into q weights on host
    out[h*128+d, s] = softmax_j(sim)^T V  computed as O^T = V_jd^T @ expS_T / denom

Sharding: data-parallel over batch, 2 batches per core on 8 cores. No collectives.
All matmuls bf16 (fp8 DoubleRow measured 2.3e-2 rel err on HW — over the 2e-2
gate — despite 1.7e-2 in CoreSim; reverted).

Schedule (v6): one long software pipeline over 8 head-units (2 batches x 4
heads). The attention inner loop is ACT-bound (exp of a [128,1024] tile is
~1.11us vs ~0.86us of PE work per j-tile), so the QKV projection matmuls are
chopped into ~0.9us half-tile units and pumped just-in-time into the head
windows as PE filler, next-head m-tiles in EARLY pump slots so their
evacuations land before the next window's S matmuls. Weight columns are
PERMUTED host-side so the wave tiles (Q0, K0, V, K1) are contiguous; the
wave-critical DMA is interleaved kt-chunk by kt-chunk across the SP and ACT
DGE queues (per-queue DMA sustains only ~110-170 GB/s) and the wave tiles
accumulate chunk-by-chunk as DMA lands, while ~24 junk matmuls on the ones
tile pre-warm the HAM clock gate. K' = K + embT is computed on the tensor
engine by an identity-matmul accumulation step (a 2-input DVE op with a PSUM
operand measures ~2x its modeled cost). The softmax denominator is a bf16 add
tree split across DVE (e0+e1+e2, joins) and GPSIMD (e3..e6), reduced across
partitions AND broadcast back to 128 rows by one ones[128,128] matmul per
512-col half; the previous head's normalize chain is emitted at j=3 of the
next window, after its GPSIMD chain has drained. The final head runs an
all-DVE tree over e0..e5 and folds e6/e7 in via extra accumulating
ones-matmuls, pipelining its recip/mult/DMA tail in halves across both DGE
queues.
"""
import numpy as np
import ml_dtypes

import concourse.bass as bass
import concourse.mybir as mybir
from concourse import bacc
from concourse import masks
import concourse.tile as tile

F32 = mybir.dt.float32
BF16 = mybir.dt.bfloat16
AF = mybir.ActivationFunctionType

B = 2          # batches per core
HEADS = 4
D = 128
S = 1024       # 32*32 spatial
C = 512        # input channels
CT = C // 128  # contraction tiles
JT = S // 128  # j tiles
NH = S // 512  # free-dim halves

# host-side weight column permutation: wave tiles (Q0, K0, V, K1) first so
# contiguous 0:896 col slices cover the warm-up wave's needs.
# m index: 0..3 = Q head m, 4..7 = K head m-4.
MCOL = {0: 0, 4: 128, 5: 768, 1: 896, 2: 1024, 6: 1152, 3: 1280, 7: 1408}
VCOL = 256  # V occupies permuted cols 256:768

_CACHED_NC = None
LAST_RESULT = None


def build():
    nc = bacc.Bacc()
    fmap_ext = nc.declare_dram_parameter("fmap", [B, C, S], BF16, isOutput=False)
    w_ext = nc.declare_dram_parameter("w_lhsT", [C, 1536], BF16, isOutput=False)
    embT_ext = nc.declare_dram_parameter("embT", [D, S], BF16, isOutput=False)
    out_ext = nc.declare_dram_parameter("out", [B, HEADS * D, S], F32, isOutput=True)

    with tile.TileContext(nc) as tc:
        with (
            tc.tile_pool(name="const", bufs=1) as const,
            tc.tile_pool(name="xp", bufs=2) as xp,
            tc.tile_pool(name="qp", bufs=2) as qp,
            tc.tile_pool(name="kp", bufs=2) as kp,
            tc.tile_pool(name="vp", bufs=2) as vp,
            tc.tile_pool(name="ep", bufs=8) as ep,
            tc.tile_pool(name="sump", bufs=8) as sump,
            tc.tile_pool(name="dp", bufs=2) as dp,
            tc.tile_pool(name="op", bufs=3) as op,
            tc.tile_pool(name="orp", bufs=2) as orp,
            tc.tile_pool(name="sp", bufs=2, space="PSUM") as sp,
            tc.tile_pool(name="fp", bufs=2, space="PSUM") as fp,
            tc.tile_pool(name="otp", bufs=1, space="PSUM") as otp,
        ):
            # ---- constants built on otherwise-idle engines ----
            ident = const.tile([128, 128], BF16)
            masks.make_identity(nc, ident[:])
            ones_sq = const.tile([128, 128], BF16)
            nc.gpsimd.memset(ones_sq[:], 1.0)

            # ---- HAM warm-up: junk matmuls on the ones tile keep the PE
            # busy during the input-DMA wait so the clock gate opens early.
            junk = otp.tile([128, S], F32, tag="ot", name="junk")
            for _ in range(24):
                nc.tensor.matmul(junk[:, 0:128], ones_sq[:], ones_sq[:],
                                 start=True, stop=True)

            # ---- SBUF-resident inputs; wave-critical tensors interleaved
            # kt-chunk by kt-chunk across the two HWDGE queues (per-queue DMA
            # sustains only ~110-170 GB/s).
            w_sb = const.tile([128, CT, 1536], BF16)
            src_w = w_ext.rearrange("(t p) o -> p t o", p=128)
            embT_sb = const.tile([D, S], BF16)
            x_sb = [None] * B
            for b in range(B):
                x_sb[b] = xp.tile([128, CT, S], BF16, tag="x", name=f"x{b}")
            xre = [fmap_ext[b].rearrange("(t p) s -> p t s", p=128)
                   for b in range(B)]
            # SP queue: wave w/x chunks interleaved, then rest
            nc.sync.dma_start(out=w_sb[:, 0, 0:896], in_=src_w[:, 0, 0:896])
            nc.sync.dma_start(out=x_sb[0][:, 1, :], in_=xre[0][:, 1, :])
            nc.sync.dma_start(out=w_sb[:, 2, 0:896], in_=src_w[:, 2, 0:896])
            nc.sync.dma_start(out=x_sb[0][:, 3, :], in_=xre[0][:, 3, :])
            nc.sync.dma_start(out=w_sb[:, :, 896:1536], in_=src_w[:, :, 896:1536])
            nc.sync.dma_start(out=x_sb[1][:, 0:2, :], in_=xre[1][:, 0:2, :])
            # ACT queue: embT first (gates the wave's identity matmuls)
            nc.scalar.dma_start(out=embT_sb, in_=embT_ext[:])
            nc.scalar.dma_start(out=x_sb[0][:, 0, :], in_=xre[0][:, 0, :])
            nc.scalar.dma_start(out=w_sb[:, 1, 0:896], in_=src_w[:, 1, 0:896])
            nc.scalar.dma_start(out=x_sb[0][:, 2, :], in_=xre[0][:, 2, :])
            nc.scalar.dma_start(out=w_sb[:, 3, 0:896], in_=src_w[:, 3, 0:896])
            nc.scalar.dma_start(out=x_sb[1][:, 2:4, :], in_=xre[1][:, 2:4, :])

            q_sb = [None] * B
            k_sb = [None] * B
            v_sb = [None] * B
            for b in range(B):
                q_sb[b] = qp.tile([128, HEADS, S], BF16, tag="q", name=f"q{b}")
                k_sb[b] = kp.tile([128, HEADS, S], BF16, tag="k", name=f"k{b}")
                v_sb[b] = vp.tile([128, JT, 512], BF16, tag="v", name=f"v{b}")

            # ---- QKV building blocks ----
            def qk_evac(b, m, p, sl=slice(0, S), eng="v"):
                if m < 4:
                    if eng == "a":
                        nc.scalar.activation(out=q_sb[b][:, m, sl], in_=p[:],
                                             func=AF.Copy)
                    else:
                        nc.vector.tensor_copy(q_sb[b][:, m, sl], p[:])
                else:
                    nc.vector.tensor_copy(k_sb[b][:, m - 4, sl], p[:])

            def qk_half(b, m, n, eng="v"):
                """Half of a QK m-tile: 4 contraction MMs (+identity embT add
                for K tiles) into one [128,512] PSUM banklet, then evac."""
                sl = slice(n * 512, (n + 1) * 512)
                p = fp.tile([128, 512], F32, tag="f", name=f"qk{b}{m}{n}")
                is_k = m >= 4
                wc = MCOL[m]
                for kt in range(CT):
                    nc.tensor.matmul(
                        p[:],
                        w_sb[:, kt, wc:wc + 128],
                        x_sb[b][:, kt, sl],
                        start=(kt == 0),
                        stop=(kt == CT - 1 and not is_k),
                    )
                if is_k:
                    # K' = K + embT via identity matmul: keeps the embT add on
                    # the PE instead of a slow PSUM-operand DVE add
                    nc.tensor.matmul(
                        p[:], ident[:], embT_sb[:, sl], start=False, stop=True,
                    )
                qk_evac(b, m, p, sl, eng)

            def v_single(b, j, p=None):
                """One V j-tile: 4 contraction MMs into [128,512], then evac."""
                if p is None:
                    p = fp.tile([128, 512], F32, tag="f", name=f"v{b}{j}")
                    for kt in range(CT):
                        nc.tensor.matmul(
                            p[:],
                            x_sb[b][:, kt, j * 128:(j + 1) * 128],
                            w_sb[:, kt, VCOL:VCOL + 512],
                            start=(kt == 0),
                            stop=(kt == CT - 1),
                        )
                nc.vector.tensor_copy(v_sb[b][:, j, :], p[:])

            # ---- pre-phase: kt-chunk-gated wave for batch 0 ----
            # (K0, Q0, K1) m-tiles + V0/V1 accumulate chunk by chunk as DMA
            # lands. K1 borrows the (junk-warmed) otp bank. K0 first so its
            # DVE evac overlaps Q0's ACT evac and S(h0, j<4) can start on the
            # first K half.
            pk0 = sp.tile([128, S], F32, tag="s", name="pk0")
            pq0 = sp.tile([128, S], F32, tag="s", name="pq0")
            pk1 = otp.tile([128, S], F32, tag="ot", name="pk1")
            pv0 = fp.tile([128, 512], F32, tag="f", name="pv0")
            pv1 = fp.tile([128, 512], F32, tag="f", name="pv1")
            for kt in range(CT):
                st, last = (kt == 0), (kt == CT - 1)
                for p, wc in ((pk0, MCOL[4]), (pq0, MCOL[0]), (pk1, MCOL[5])):
                    for n in range(NH):
                        nc.tensor.matmul(
                            p[:, n * 512:(n + 1) * 512],
                            w_sb[:, kt, wc:wc + 128],
                            x_sb[0][:, kt, n * 512:(n + 1) * 512],
                            start=st, stop=(last and p is pq0),
                        )
                for j, pv in ((0, pv0), (1, pv1)):
                    nc.tensor.matmul(
                        pv[:],
                        x_sb[0][:, kt, j * 128:(j + 1) * 128],
                        w_sb[:, kt, VCOL:VCOL + 512],
                        start=st, stop=last,
                    )
            for p in (pk0, pk1):
                for n in range(NH):
                    nc.tensor.matmul(
                        p[:, n * 512:(n + 1) * 512],
                        ident[:], embT_sb[:, n * 512:(n + 1) * 512],
                        start=False, stop=True,
                    )
            # K0 evac in halves: S(h0, j=0..3) only needs half 0
            nc.vector.tensor_copy(k_sb[0][:, 0, 0:512], pk0[:, 0:512])
            nc.vector.tensor_copy(k_sb[0][:, 0, 512:1024], pk0[:, 512:1024])
            qk_evac(0, 0, pq0, eng="a")   # ACT is idle pre-h0
            qk_evac(0, 5, pk1)
            v_single(0, 0, p=pv0)
            v_single(0, 1, p=pv1)

            # ---- filler schedule ----
            # fillers[w][j] = units pumped right before PV(j) of window w.
            # V(b, j) must be evacuated before that batch's first head runs
            # PV(j) (the PV stream lags the exps by ~1.5us, which the V pump
            # slots exploit); next-head m-tiles sit in EARLY pump slots so
            # their evacs land before the next window's S matmuls.
            Q = lambda b, m, n: ("q", b, m, n)
            Qa = lambda b, m, n: ("qa", b, m, n)   # ACT-evac variant
            V = lambda b, j: ("v", b, j)
            JU = ("junk",)
            # The exp stream is the backbone: any filler in a window's LAST
            # j-slots sits in the PE FIFO ahead of the next window's S(0) and
            # breaks the exp stream at the boundary. From h3 on (where ACT
            # pacing binds), fillers go in slots j=0..3 of the FOLLOWING
            # window instead; h0-h2 (PE-bound ramp) absorb the overflow.
            # Junk units keep the HAM clock gate warm through the thin late
            # windows (a >=2us PE gap risks a 3.4us half-clock re-throttle).
            fillers = [
                # h0 (b0h0): b0 V tiles JIT + Q1 halves for b0h1
                {0: [V(0, 2), Qa(0, 1, 0)], 1: [V(0, 3), Qa(0, 1, 1)],
                 2: [V(0, 4)], 3: [V(0, 5)], 4: [V(0, 6)], 5: [V(0, 7)]},
                # h1 (b0h1): b0h2 m-tiles + b1h0 Q halves (overflow, late ok)
                {0: [Qa(0, 2, 0)], 1: [Qa(0, 2, 1)], 2: [Q(0, 6, 0)],
                 3: [Q(0, 6, 1)], 4: [Qa(1, 0, 0)], 5: [Qa(1, 0, 1)]},
                # h2 (b0h2): b0h3 m-tiles + b1h0 K halves (overflow)
                {0: [Qa(0, 3, 0)], 1: [Qa(0, 3, 1)], 2: [Q(0, 7, 0)],
                 3: [Q(0, 7, 1)], 4: [Q(1, 4, 0)], 5: [Q(1, 4, 1)]},
                # h3 (b0h3): b1h1 Q halves early, first b1 V tiles
                {0: [Qa(1, 1, 0)], 1: [Qa(1, 1, 1)], 2: [V(1, 0)],
                 3: [V(1, 1)], 4: [V(1, 2)], 5: [V(1, 3)]},
                # h4 (b1h0): b1h1 K halves early, rest of b1 V JIT
                {0: [Q(1, 5, 0)], 1: [Q(1, 5, 1)], 2: [V(1, 4)],
                 3: [V(1, 5)], 4: [V(1, 6)], 5: [V(1, 7)]},
                # h5 (b1h1): b1h2 m-tiles, early slots only
                {0: [Qa(1, 2, 0)], 1: [Qa(1, 2, 1)], 2: [Q(1, 6, 0)],
                 3: [Q(1, 6, 1)], 4: [JU]},
                # h6 (b1h2): b1h3 m-tiles, early slots only
                {0: [Qa(1, 3, 0)], 1: [Qa(1, 3, 1)], 2: [Q(1, 7, 0)],
                 3: [Q(1, 7, 1)], 4: [JU]},
                # h7 (b1h3): junk warmth only
                {4: [JU]},
            ]

            def emit_filler(unit):
                if unit[0] == "q":
                    qk_half(*unit[1:])
                elif unit[0] == "qa":
                    qk_half(*unit[1:], eng="a")
                elif unit[0] == "junk":
                    p = fp.tile([128, 512], F32, tag="f", name="junkf")
                    for _ in range(9):
                        nc.tensor.matmul(p[:, 0:128], ones_sq[:], ones_sq[:],
                                         start=True, stop=True)
                else:
                    v_single(*unit[1:])

            # ---- attention head window ----
            def emit_s(b, h, j):
                s_ps = sp.tile([128, S], F32, tag="s", name="s_ps")
                for n in range(NH):
                    nc.tensor.matmul(
                        s_ps[:, n * 512:(n + 1) * 512],
                        k_sb[b][:, h, j * 128:(j + 1) * 128],
                        q_sb[b][:, h, n * 512:(n + 1) * 512],
                        start=True, stop=True,
                    )
                return s_ps

            def emit_head(widx, b, h, pend_evac=None, pend_norm=None):
                """Returns (ot_ps, tree_state) for the tail.

                The previous head's tail is emitted in two pieces: the O^T
                evacuation right after this window's first two S groups (so
                the single otp buffer frees before PV(j=0) here), and the
                normalize chain at j=3 (by then the previous head's GPSIMD
                denominator chain has drained, so its broadcast matmul slots
                between S/PV groups without stalling the PE).
                """
                pump = fillers[widx]
                last_head = widx == B * HEADS - 1
                ot_ps = otp.tile([128, S], F32, tag="ot", name="ot_ps")
                s_tiles = [emit_s(b, h, 0), emit_s(b, h, 1)]
                if pend_evac is not None:
                    pend_evac()
                exps = [None] * JT
                acc = None   # DVE accumulator
                gcc = None   # GPSIMD accumulator (e3..e6), heads 0..6 only
                for j in range(JT):
                    e = ep.tile([128, S], BF16, tag="exps", name="exps")
                    exps[j] = e
                    nc.scalar.activation(out=e[:], in_=s_tiles[j][:], func=AF.Exp)
                    if j + 2 < JT:
                        s_tiles.append(emit_s(b, h, j + 2))
                    if j == 3 and pend_norm is not None:
                        pend_norm()
                    for unit in pump.get(j, ()):
                        emit_filler(unit)
                    for n in range(NH):
                        nc.tensor.matmul(
                            ot_ps[:, n * 512:(n + 1) * 512],
                            v_sb[b][:, j, h * 128:(h + 1) * 128],
                            e[:, n * 512:(n + 1) * 512],
                            start=(j == 0),
                            stop=(j == JT - 1),
                        )
                    # denominator add tree: heads 0..6: DVE takes e0+e1+e2
                    # and the joins, GPSIMD (otherwise idle, but ~2.4us/add)
                    # chains e3..e6. The last head runs an all-DVE chain over
                    # e0..e5 and folds e6/e7 in via extra accumulating
                    # ones-matmuls in the tail (shortest exposed latency).
                    if j == 1:
                        acc = sump.tile([128, S], BF16, tag="tree", name="acc")
                        nc.vector.tensor_add(acc[:], exps[0][:], exps[1][:])
                    elif j == 2:
                        nc.vector.tensor_add(acc[:], acc[:], e[:])
                    elif j in (3, 4, 5) and last_head:
                        nc.vector.tensor_add(acc[:], acc[:], e[:])
                    elif j == 4 and not last_head:
                        gcc = sump.tile([128, S], BF16, tag="tree", name="gcc")
                        nc.gpsimd.tensor_add(gcc[:], exps[3][:], exps[4][:])
                    elif j in (5, 6) and not last_head:
                        nc.gpsimd.tensor_add(gcc[:], gcc[:], e[:])
                if last_head:
                    return ot_ps, (acc, exps[6], exps[7])
                t7 = sump.tile([128, S], BF16, tag="tree", name="t7")
                nc.vector.tensor_add(t7[:], acc[:], exps[7][:])
                expsum = sump.tile([128, S], BF16, tag="tree", name="expsum")
                nc.vector.tensor_add(expsum[:], t7[:], gcc[:])
                return ot_ps, expsum

            def emit_tail_evac(widx, ot_ps):
                # evacuate O^T promptly so otp frees for the next head's PV;
                # late windows have no PE filler so ACT only helps early ones
                o_raw = orp.tile([128, S], F32, tag="oraw", name="o_raw")
                if widx < 4 or widx == B * HEADS - 1:
                    # ACT helps in the PE-bound ramp, and is free after the
                    # final head's last exp
                    nc.scalar.activation(out=o_raw[:, 0:512], in_=ot_ps[:, 0:512],
                                         func=AF.Copy)
                else:
                    nc.vector.tensor_copy(o_raw[:, 0:512], ot_ps[:, 0:512])
                nc.vector.tensor_copy(o_raw[:, 512:1024], ot_ps[:, 512:1024])
                return o_raw

            def emit_tail_norm(b, h, o_raw, tree, last):
                # denominator reduce + broadcast in ONE matmul per half:
                # b_ps[p, i] = sum_j ones[j, p] * expsum[j, i]
                srcs = (tree,) if not last else tree
                b_ps = [None] * NH
                for n in range(NH):
                    b_ps[n] = fp.tile([128, 512], F32, tag="f", name=f"b_ps{n}")
                    for si, src in enumerate(srcs):
                        nc.tensor.matmul(
                            b_ps[n][:], ones_sq[:],
                            src[:, n * 512:(n + 1) * 512],
                            start=(si == 0), stop=(si == len(srcs) - 1),
                        )
                if not last:
                    bsb = dp.tile([128, S], F32, tag="bsb", name="bsb")
                    for n in range(NH):
                        nc.vector.reciprocal_approx_fast(
                            bsb[:, n * 512:(n + 1) * 512], b_ps[n][:])
                    o_sb = op.tile([128, S], F32, tag="o", name="o_sb")
                    nc.vector.tensor_tensor(
                        out=o_sb[:], in0=o_raw[:], in1=bsb[:],
                        op=mybir.AluOpType.mult,
                    )
                    nc.sync.dma_start(
                        out=out_ext[b, h * 128:(h + 1) * 128, :], in_=o_sb[:]
                    )
                else:
                    # final head: pipeline recip/mult/DMA in 256-col quarters,
                    # output DMA alternating across the SP and ACT DGE queues.
                    # GPSIMD (idle at kernel end, SBUF-only operands) takes
                    # the odd mult quarters so the DVE chain shortens.
                    for n in range(2 * NH):
                        sl = slice(n * 256, (n + 1) * 256)
                        hsl = slice((n % 2) * 256, (n % 2) * 256 + 256)
                        bsb = dp.tile([128, 256], F32, tag=f"bl{n}", name="bsbl")
                        nc.vector.reciprocal_approx_fast(
                            bsb[:], b_ps[n // 2][:, hsl])
                        o_sb = op.tile([128, 256], F32, tag=f"ol{n}", name="o_l")
                        meng = nc.vector if n % 2 == 0 else nc.gpsimd
                        meng.tensor_tensor(
                            out=o_sb[:], in0=o_raw[:, sl], in1=bsb[:],
                            op=mybir.AluOpType.mult,
                        )
                        eng = nc.sync if n % 2 == 0 else nc.scalar
                        eng.dma_start(
                            out=out_ext[b, h * 128:(h + 1) * 128, sl], in_=o_sb[:]
                        )

            # ---- main loop: 8 head windows, tails pipelined one behind ----
            units = [(b, h) for b in range(B) for h in range(HEADS)]
            pend = None
            box = []
            for widx, (b, h) in enumerate(units):
                if pend is not None:
                    pw, pb, ph, pot, ptree = pend
                    evac = lambda pw=pw, pot=pot: box.append(
                        emit_tail_evac(pw, pot))
                    norm = lambda pb=pb, ph=ph, ptree=ptree: emit_tail_norm(
                        pb, ph, box.pop(), ptree, last=False)
                else:
                    evac = norm = None
                state = emit_head(widx, b, h, pend_evac=evac, pend_norm=norm)
                pend = (widx, b, h, *state)
            pw, pb, ph, pot, ptree = pend
            o_raw_last = emit_tail_evac(pw, pot)
            emit_tail_norm(pb, ph, o_raw_last, ptree, last=True)
    nc.finalize()
    return nc


def _get_nc():
    global _CACHED_NC
    if _CACHED_NC is None:
        _CACHED_NC = build()
    return _CACHED_NC


def kernel(fmap, w_qkv, height, width):
    fmap = np.ascontiguousarray(np.asarray(fmap, dtype=np.float32))
    w_qkv = np.asarray(w_qkv, dtype=np.float32)
    height = np.asarray(height, dtype=np.float32)
    width = np.asarray(width, dtype=np.float32)

    nb, c, hh, ww = fmap.shape  # (16, 512, 32, 32)
    s = hh * ww
    scale = D ** -0.5

    w_lhsT = np.ascontiguousarray(w_qkv.T).astype(np.float32)  # [512, 1536]
    w_lhsT[:, :512] *= scale  # fold softmax scale into Q projection
    # permute columns: wave tiles (Q0, K0, V, K1) first — see MCOL/VCOL
    w_perm = np.concatenate([
        w_lhsT[:, 0:128],      # Q0 @ 0
        w_lhsT[:, 512:640],    # K0 @ 128
        w_lhsT[:, 1024:1536],  # V  @ 256
        w_lhsT[:, 640:768],    # K1 @ 768
        w_lhsT[:, 128:256],    # Q1 @ 896
        w_lhsT[:, 256:384],    # Q2 @ 1024
        w_lhsT[:, 768:896],    # K2 @ 1152
        w_lhsT[:, 384:512],    # Q3 @ 1280
        w_lhsT[:, 896:1024],   # K3 @ 1408
    ], axis=1).astype(ml_dtypes.bfloat16)
    embT = np.ascontiguousarray(
        (height[:, None, :] + width[None, :, :]).reshape(s, D).T
    ).astype(ml_dtypes.bfloat16)  # [128, 1024]

    fm = fmap.reshape(nb, c, s).astype(ml_dtypes.bfloat16)
    nc = _get_nc()
    in_maps = [
        {"fmap": fm[B * i:B * (i + 1)], "w_lhsT": w_perm, "embT": embT}
        for i in range(8)
    ]

    from concourse.bass_utils import run_bass_kernel_spmd
    res = run_bass_kernel_spmd(nc, in_maps, core_ids=list(range(8)))
    global LAST_RESULT
    LAST_RESULT = res
    out = np.concatenate([r["out"] for r in res.results], axis=0)  # (16, 512, 1024)
    return np.ascontiguousarray(out.reshape(nb, HEADS * D, hh, ww)).astype(np.float32)


if __name__ == "__main__":
    rng = np.random.default_rng(0)
    inputs = {
        "fmap": rng.standard_normal((16, 512, 32, 32)).astype(np.float32),
        "w_qkv": (rng.standard_normal((1536, 512)) * 0.02).astype(np.float32),
        "height": (rng.standard_normal((32, 128)) * (128 ** -0.5)).astype(np.float32),
        "width": (rng.standard_normal((32, 128)) * (128 ** -0.5)).astype(np.float32),
    }
    out = kernel(**inputs)
    print(out.shape, out.dtype)
